# revision 1
# baseline (speedup 1.0000x reference)
"""Causal MHA with RoPE on 8 Trainium2 NeuronCores.

Sharding: core c -> batch b=c//2, head-group g=c%2 (8 heads of 16).
Each core: Q/K/V projections for its 512 head-dims over the full sequence,
causal attention for its 8 heads, partial output projection (its 512 rows
of wo). Host sums the two partial outputs per batch. No collectives.

All matmuls run in float32r (full-rate PE mode, ~1.5e-4 rel err at K=1024);
projections, attention (query-block qb==pass) and the output projection are
interleaved across four 512-token passes so exp (ScalarE) overlaps PE work.
Timeline-sim estimate ~413us/core; measured rel err vs fp32 reference 2.6e-4.
RoPE is applied via host-permuted wq/wk columns ([evens|odds] per head),
32-row block-swap DMAs and precomputed cos/sin tables.
Softmax skips max-subtraction (scores are O(1) after the 1/8 scale), uses
an additive -1e9 causal mask on diagonal tiles, and gets denominators from
a ones-column appended to V (M=65 AV matmul).
"""

import math

import numpy as np

import concourse.bass as bass
import concourse.mybir as mybir
import concourse.tile as tile
from concourse import bacc
from concourse.bass_utils import run_bass_kernel_spmd
from concourse.masks import make_identity

F32 = mybir.dt.float32
F32R = mybir.dt.float32r

B, S, D, H = 4, 2048, 1024, 16
HD = D // H          # 64
THETA = 10000.0
DH = D // 2          # 512 per-core head dims (8 heads)
NP = 4               # head pairs per core
NTH = 4              # token passes for x^T materialization / projections
THT = S // NTH       # 512 tokens per pass
NQB = 4              # query blocks of 512
QB = S // NQB
NKT = S // 128       # 16 key tiles of 128
SCALE = 1.0 / math.sqrt(HD)
NEG = -1.0e9

_cached = None


def _build():
    nc = bacc.Bacc(None, target_bir_lowering=False)

    x = nc.dram_tensor("x", [S, D], F32, kind="ExternalInput")
    wk = nc.dram_tensor("wk", [D, DH], F32, kind="ExternalInput")
    wq = nc.dram_tensor("wq", [D, DH], F32, kind="ExternalInput")
    wv = nc.dram_tensor("wv", [D, DH], F32, kind="ExternalInput")
    wo = nc.dram_tensor("wo", [DH, D], F32, kind="ExternalInput")
    cosb = nc.dram_tensor("cosb", [128, S], F32, kind="ExternalInput")
    sinb = nc.dram_tensor("sinb", [128, S], F32, kind="ExternalInput")
    outp = nc.dram_tensor("outp", [S, D], F32, kind="ExternalOutput")

    with tile.TileContext(nc) as tc:
        with (
            tc.tile_pool(name="const", bufs=1) as cpool,
            tc.tile_pool(name="kq", bufs=1) as kqpool,
            tc.tile_pool(name="vaug", bufs=1) as vpool,
            tc.tile_pool(name="xt", bufs=8) as xtpool,
            tc.tile_pool(name="stream", bufs=2) as spool,
            tc.tile_pool(name="w512", bufs=8) as wpool,
        ):
            ident = cpool.tile([128, 128], F32, name="ident")
            make_identity(nc, ident)
            tri = cpool.tile([128, 128], F32, name="tri")
            nc.gpsimd.memset(tri, 0.0)
            # tri[i, t] = 0 if t >= i else NEG  (mask k>q inside diagonal tiles)
            nc.gpsimd.affine_select(
                out=tri, in_=tri, compare_op=mybir.AluOpType.is_ge,
                fill=NEG, base=0, pattern=[[1, 128]], channel_multiplier=-1,
            )
            cos_t = cpool.tile([128, S], F32, name="cos_t")
            sin_t = cpool.tile([128, S], F32, name="sin_t")
            nc.sync.dma_start(out=cos_t, in_=cosb[:, :])
            nc.sync.dma_start(out=sin_t, in_=sinb[:, :])

            # K^T / Q^T pair tiles: [128 dims (head 2p | head 2p+1), S tokens]
            kt_tiles = [kqpool.tile([128, S], F32R, name=f"ktp{p}", tag=f"ktp{p}") for p in range(NP)]
            qt_tiles = [kqpool.tile([128, S], F32R, name=f"qtp{p}", tag="qc", bufs=8) for p in range(NP)]
            ctx_tiles = [kqpool.tile([128, S], F32R, name=f"ctxp{p}", tag="qc", bufs=8) for p in range(NP)]
            # V tiles with ones column: [128 tokens, 8 heads, 64+1]
            v_tiles = [vpool.tile([128, 8, HD + 1], F32R, name=f"vt{t}", tag=f"vt{t}") for t in range(NKT)]
            for t in range(NKT):
                # ones column via exp(0*x) = 1
                nc.scalar.activation(
                    v_tiles[t][:, :, HD], cos_t[:, 0:8],
                    mybir.ActivationFunctionType.Exp, scale=0.0,
                )

            with (
                tc.tile_pool(name="pst", bufs=2, space="PSUM") as pst,
                tc.tile_pool(name="pssc", bufs=2, space="PSUM") as pssc,
                tc.tile_pool(name="psc", bufs=1, space="PSUM") as psc,
            ):
                for th in range(NTH):
                    t0 = th * THT
                    # ---- x^T materialization for this token pass ----
                    xtb = [xtpool.tile([128, 4, THT], F32R, name=f"xtb{th}_{h}", tag="xt", bufs=2)
                           for h in range(2)]
                    xts = [xtb[dc // 4][:, dc % 4, :] for dc in range(8)]
                    for tl in range(THT // 128):
                        for hf in range(2):
                            xl = spool.tile([128, D // 2], F32, name="xl", tag="xl")
                            nc.sync.dma_start(
                                out=xl,
                                in_=x[t0 + tl * 128 : t0 + (tl + 1) * 128,
                                      hf * 512 : (hf + 1) * 512])
                            tp = pst.tile([128, 512], F32, name="tp", tag="tp")
                            for dq in range(4):
                                nc.tensor.matmul(
                                    tp[:, dq * 128 : (dq + 1) * 128],
                                    xl[:, dq * 128 : (dq + 1) * 128], ident,
                                    is_transpose=True,
                                    start=(dq == 0), stop=(dq == 3))
                            nc.vector.tensor_copy(
                                xtb[hf][:, :, tl * 128 : (tl + 1) * 128],
                                tp.rearrange("a (c d) -> a c d", c=4))

                    # ---- K^T / Q^T projections + RoPE for this token pass ----
                    for wmat, dst in ((wk, kt_tiles), (wq, qt_tiles)):
                        wmr = wmat.rearrange("(c p) j -> p c j", p=128)
                        wbh = []
                        for h in range(2):
                            wb = wpool.tile([128, 4, DH], F32R, name="wb", tag="wbig", bufs=2)
                            nc.gpsimd.dma_start(out=wb, in_=wmr[:, 4 * h : 4 * h + 4, :])
                            wbh.append(wb)
                        for p in range(NP):
                            acc = pst.tile([128, THT], F32, name="acc", tag="tp")
                            for dc in range(8):
                                nc.tensor.matmul(
                                    acc, wbh[dc // 4][:, dc % 4, p * 128 : (p + 1) * 128], xts[dc],
                                    start=(dc == 0), stop=(dc == 7),
                                )
                            # rope fused with psum evacuation:
                            #   dst = acc*C - swap(acc*S)   (C swap-symmetric, S anti-symmetric)
                            dslice = dst[p][:, t0 : t0 + THT]
                            nc.vector.tensor_mul(dslice, acc, cos_t[:, t0 : t0 + THT])
                            raw = spool.tile([128, THT], F32, name="raw", tag="raw", bufs=2)
                            nc.vector.tensor_mul(raw, acc, sin_t[:, t0 : t0 + THT])
                            swp = spool.tile([128, THT], F32, name="swp", tag="swp", bufs=1)
                            nc.sync.dma_start(out=swp[0:32, :], in_=raw[32:64, :])
                            nc.sync.dma_start(out=swp[32:64, :], in_=raw[0:32, :])
                            nc.sync.dma_start(out=swp[64:96, :], in_=raw[96:128, :])
                            nc.sync.dma_start(out=swp[96:128, :], in_=raw[64:96, :])
                            nc.vector.tensor_sub(dslice, dslice, swp)

                    # ---- V projection for this token pass ----
                    wvr_ = wv.rearrange("(c p) j -> p c j", p=128)
                    wvbh = []
                    for h in range(2):
                        wvb = wpool.tile([128, 4, DH], F32R, name="wvb", tag="wbig", bufs=2)
                        nc.gpsimd.dma_start(out=wvb, in_=wvr_[:, 4 * h : 4 * h + 4, :])
                        wvbh.append(wvb)
                    for tl in range(THT // 128):
                        acc = pst.tile([128, DH], F32, name="vacc", tag="tp")
                        for dc in range(8):
                            nc.tensor.matmul(
                                acc, xts[dc][:, tl * 128 : (tl + 1) * 128],
                                wvbh[dc // 4][:, dc % 4, :],
                                start=(dc == 0), stop=(dc == 7),
                            )
                        vt = v_tiles[th * (THT // 128) + tl]
                        # strided write: psum [128, 8*64] -> v_aug[:, h, 0:64]
                        nc.vector.tensor_copy(
                            vt[:, :, 0:HD],
                            acc.rearrange("a (h d) -> a h d", h=8),
                        )

                    # ---- attention for query block qb == th (all pairs) ----
                    qb = th
                    for p in range(NP):
                        ktp, qtp = kt_tiles[p], qt_tiles[p]
                        q0 = qb * QB
                        nk = 4 * qb + 4
                        pse = psc.tile([HD + 1, QB], F32, name="pse", tag="ctxe")
                        pso = psc.tile([HD + 1, QB], F32, name="pso", tag="ctxo")
                        for kt in range(nk):
                            dj = kt - (nk - 4)
                            qoff = 128 * dj if dj > 0 else 0
                            n = QB - qoff
                            psab = pssc.tile([128, 2 * QB], F32, name="psab", tag="sc")
                            ksl = slice(kt * 128, (kt + 1) * 128)
                            qsl = slice(q0 + qoff, q0 + QB)
                            nc.tensor.matmul(psab[:, 0:n], ktp[0:64, ksl], qtp[0:64, qsl])
                            nc.tensor.matmul(psab[:, QB : QB + n], ktp[64:128, ksl], qtp[64:128, qsl])
                            if dj >= 0:
                                mview = bass.AP(
                                    tensor=psab.tensor, offset=psab.offset,
                                    ap=[psab.ap[0], [QB, 2], [1, 128]])
                                tview = bass.AP(
                                    tensor=tri.tensor, offset=tri.offset,
                                    ap=[tri.ap[0], [0, 2], [1, 128]])
                                nc.vector.tensor_add(mview, mview, tview)
                            eab = spool.tile([128, 2 * QB], F32R, name="eab", tag="eab")
                            eview_o = bass.AP(
                                tensor=eab.tensor, offset=eab.offset,
                                ap=[eab.ap[0], [QB, 2], [1, n]])
                            eview_i = bass.AP(
                                tensor=psab.tensor, offset=psab.offset,
                                ap=[psab.ap[0], [QB, 2], [1, n]])
                            nc.scalar.activation(
                                eview_o, eview_i, mybir.ActivationFunctionType.Exp, scale=SCALE)
                            vt = v_tiles[kt]
                            nc.tensor.matmul(
                                pse[:, qoff:QB], vt[:, 2 * p, :], eab[:, 0:n],
                                start=(kt == 0), stop=(kt == nk - 1))
                            nc.tensor.matmul(
                                pso[:, qoff:QB], vt[:, 2 * p + 1, :], eab[:, QB : QB + n],
                                start=(kt == 0), stop=(kt == nk - 1))
                        for par, psx in ((0, pse), (1, pso)):
                            s0 = spool.tile([1, QB], F32, name="s0", tag="s0", bufs=2)
                            nc.vector.reciprocal(s0[0:1, :], psx[HD : HD + 1, :])
                            bc = spool.tile([HD, QB], F32, name="bc", tag="bc", bufs=2)
                            nc.gpsimd.partition_broadcast(bc, s0[0:1, :])
                            nc.vector.tensor_mul(
                                ctx_tiles[p][par * HD : (par + 1) * HD, q0 : q0 + QB],
                                psx[0:HD, :], bc)


                # ---------------- output projection ----------------
                wo_r = wo.rearrange("(c p) j -> p c j", p=128)
                for nn in range(2):
                    wob = wpool.tile([128, 4, 512], F32R, name="wob", tag="wbig", bufs=2)
                    nc.gpsimd.dma_start(out=wob, in_=wo_r[:, :, nn * 512 : (nn + 1) * 512])
                    for t in range(NKT):
                        acc = pst.tile([128, 512], F32, name="oacc", tag="tp")
                        for pc in range(4):
                            nc.tensor.matmul(
                                acc, ctx_tiles[pc][:, t * 128 : (t + 1) * 128],
                                wob[:, pc, :], start=(pc == 0), stop=(pc == 3))
                        osb = spool.tile([128, 512], F32, name="osb", tag="osb", bufs=2)
                        nc.scalar.copy(osb, acc)
                        nc.sync.dma_start(
                            out=outp[t * 128 : (t + 1) * 128, nn * 512 : (nn + 1) * 512], in_=osb)

    nc.compile()
    return nc


def _host_tables(token_positions):
    pos = np.asarray(token_positions, dtype=np.float64)
    inv_freq = np.exp(np.arange(0, HD, 2, dtype=np.float64) * (-math.log(THETA) / HD))  # [32]
    ang = pos[:, None] * inv_freq[None, :]  # [S, 32]
    cos = np.cos(ang).astype(np.float32).T  # [32, S]
    sin = np.sin(ang).astype(np.float32).T
    # pair-tile row layout: [head_even: 32 evens | 32 odds][head_odd: same]
    C = np.empty((128, S), np.float32)
    Sx = np.empty((128, S), np.float32)
    for half in range(2):
        r0 = 64 * half
        C[r0 : r0 + 32] = cos
        C[r0 + 32 : r0 + 64] = cos
        Sx[r0 : r0 + 32] = -sin
        Sx[r0 + 32 : r0 + 64] = sin
    return C, Sx


def kernel(in_features, token_positions, wq, wk, wv, wo):
    global _cached
    if _cached is None:
        _cached = _build()
    nc = _cached

    x = np.ascontiguousarray(in_features, dtype=np.float32)
    # permute wq/wk columns within each head: [evens | odds]
    perm = np.concatenate(
        [64 * h + np.concatenate([np.arange(0, 64, 2), np.arange(1, 64, 2)]) for h in range(H)])
    wqp = np.ascontiguousarray(wq[:, perm], dtype=np.float32)
    wkp = np.ascontiguousarray(wk[:, perm], dtype=np.float32)
    wv = np.ascontiguousarray(wv, dtype=np.float32)
    wo = np.ascontiguousarray(wo, dtype=np.float32)
    C, Sx = _host_tables(token_positions)

    in_maps = []
    for c in range(8):
        b, g = c // 2, c % 2
        sl = slice(g * DH, (g + 1) * DH)
        in_maps.append({
            "x": np.ascontiguousarray(x[b]),
            "wq": np.ascontiguousarray(wqp[:, sl]),
            "wk": np.ascontiguousarray(wkp[:, sl]),
            "wv": np.ascontiguousarray(wv[:, sl]),
            "wo": np.ascontiguousarray(wo[sl, :]),
            "cosb": C,
            "sinb": Sx,
        })
    results = _run(nc, in_maps)
    out = np.empty((B, S, D), np.float32)
    for b in range(B):
        out[b] = results[2 * b]["outp"] + results[2 * b + 1]["outp"]
    return out


_jit_cache = None


def _run(nc, in_maps):
    """Run the SPMD program on 8 cores, caching the jitted executable across
    calls (run_bass_kernel_spmd retraces every call). Falls back to the
    library path on any failure."""
    global _jit_cache
    try:
        import jax
        import jax.numpy as jnp
        from jax.sharding import Mesh, PartitionSpec
        from jax.experimental.shard_map import shard_map
        from concourse import bass2jax
        import concourse.mybir as mybir

        if _jit_cache is None:
            bass2jax.install_neuronx_cc_hook()
            pid_name = nc.partition_id_tensor.name if nc.partition_id_tensor else None
            in_names, out_names, out_avals, zero_outs = [], [], [], []
            for alloc in nc.m.functions[0].allocations:
                if not isinstance(alloc, mybir.MemoryLocationSet):
                    continue
                nm = alloc.memorylocations[0].name
                if alloc.kind == "ExternalInput":
                    if nm != pid_name:
                        in_names.append(nm)
                elif alloc.kind == "ExternalOutput":
                    out_names.append(nm)
                    shape = tuple(alloc.tensor_shape)
                    dtype = mybir.dt.np(alloc.dtype)
                    out_avals.append(jax.core.ShapedArray(shape, dtype))
                    zero_outs.append(np.zeros(shape, dtype))
            n_params = len(in_names)
            all_names = in_names + out_names
            if pid_name is not None:
                all_names = all_names + [pid_name]

            def _body(*args):
                operands = list(args)
                if pid_name is not None:
                    operands.append(bass2jax.partition_id_tensor())
                outs = bass2jax._bass_exec_p.bind(
                    *operands, out_avals=tuple(out_avals), in_names=tuple(all_names),
                    out_names=tuple(out_names), lowering_input_output_aliases=(),
                    sim_require_finite=True, sim_require_nnan=True, nc=nc)
                return tuple(outs)

            devices = jax.devices()[:8]
            mesh = Mesh(np.asarray(devices), ("core",))
            nio = n_params + len(out_names)
            sharded = jax.jit(
                shard_map(_body, mesh=mesh, in_specs=(PartitionSpec("core"),) * nio,
                          out_specs=(PartitionSpec("core"),) * len(out_names),
                          check_rep=False),
                keep_unused=True)
            _jit_cache = (sharded, in_names, out_names, zero_outs)

        sharded, in_names, out_names, zero_outs = _jit_cache
        concat_in = [np.concatenate([np.asarray(m[nm]) for m in in_maps], axis=0)
                     for nm in in_names]
        concat_zero = [np.concatenate([z] * 8, axis=0) for z in zero_outs]
        outs = sharded(*concat_in, *concat_zero)
        results = []
        for c in range(8):
            d = {}
            for i, nm in enumerate(out_names):
                arr = np.asarray(outs[i])
                n0 = arr.shape[0] // 8
                d[nm] = arr[c * n0 : (c + 1) * n0]
            results.append(d)
        return results
    except Exception:
        res = run_bass_kernel_spmd(nc, in_maps, core_ids=list(range(8)))
        return res.results



# revision 2
# speedup vs baseline: 1.6051x; 1.6051x over previous
"""Causal MHA with RoPE on 8 Trainium2 NeuronCores — v2 (all-bf16, flipped AV).

Sharding: core c -> batch b=c//2, head-group g=c%2 (8 heads of 16).
Each core: Q/K/V projections for its 512 head-dims over the full sequence,
causal attention for its 8 heads, partial output projection (its 512 rows
of wo). Host sums the two partial (bf16) outputs per batch. No collectives.

v2 changes vs baseline:
- All matmul operands bf16 (x, w, q, k, v, exp-scores, ctx): 1.0 cycles/row
  with no fp32r small-tile (ap<256) 4x penalty; transposes 1.0 vs 2.0.
- AV matmul flipped to out[queries, dims]: cost is charged per output
  free-element, so [128q, 65] accumulation over key tiles costs 65/pass
  instead of 128+/pass -> AV drops ~139K -> ~71K PE cycles.
- Denominators land on PSUM partitions (ones-column of V), so softmax
  normalization is reciprocal [128,1] + per-partition tensor_scalar mult.
- Causal masking post-exp via gpsimd affine_select on eab (Pool engine),
  keeping DVE free; exp runs on unmasked scores (finite, bounded).
- Output projection consumes PE-transposed ctx; out DMA'd as bf16 partials.
"""

import math

import numpy as np

import concourse.bass as bass
import concourse.mybir as mybir
import concourse.tile as tile
from concourse import bacc
from concourse.bass_utils import run_bass_kernel_spmd
from concourse.masks import make_identity

F32 = mybir.dt.float32
BF16 = mybir.dt.bfloat16

B, S, D, H = 4, 2048, 1024, 16
HD = D // H          # 64
THETA = 10000.0
DH = D // 2          # 512 per-core head dims (8 heads)
NP = 4               # head pairs per core
NTH = 4              # token passes (512 each)
THT = S // NTH       # 512
NKT = S // 128       # 16 key tiles of 128
SCALE = 1.0 / math.sqrt(HD)

_cached = None


def _build():
    nc = bacc.Bacc(None, target_bir_lowering=False)

    x = nc.dram_tensor("x", [S, D], BF16, kind="ExternalInput")
    wq = nc.dram_tensor("wq", [D, DH], BF16, kind="ExternalInput")
    wk = nc.dram_tensor("wk", [D, DH], BF16, kind="ExternalInput")
    wv = nc.dram_tensor("wv", [D, DH], BF16, kind="ExternalInput")
    wo = nc.dram_tensor("wo", [DH, D], BF16, kind="ExternalInput")
    cosb = nc.dram_tensor("cosb", [128, S], F32, kind="ExternalInput")
    sinb = nc.dram_tensor("sinb", [128, S], F32, kind="ExternalInput")
    pswap = nc.dram_tensor("pswap", [128, 128], BF16, kind="ExternalInput")
    outp = nc.dram_tensor("outp", [S, D], BF16, kind="ExternalOutput")

    with tile.TileContext(nc) as tc:
        with (
            tc.tile_pool(name="const", bufs=1) as cpool,
            tc.tile_pool(name="kq", bufs=1) as kqpool,
            tc.tile_pool(name="vaug", bufs=1) as vpool,
            tc.tile_pool(name="xt", bufs=2) as xtpool,
            tc.tile_pool(name="stream", bufs=2) as spool,
            tc.tile_pool(name="eab", bufs=1) as epool,
            tc.tile_pool(name="w0", bufs=1) as wpool,
        ):
            # resident weights; wk/wq on the HWDGE path (needed first),
            # wv/wo + tables on SWDGE in parallel
            wq_sb = wpool.tile([128, 8, DH], BF16, name="wq_sb")
            wk_sb = wpool.tile([128, 8, DH], BF16, name="wk_sb")
            wv_sb = wpool.tile([128, 8, DH], BF16, name="wv_sb")
            wo_sb = wpool.tile([128, 4, 2, DH], BF16, name="wo_sb")
            cos_t = cpool.tile([128, S], F32, name="cos_t")
            sin_t = cpool.tile([128, S], F32, name="sin_t")
            psw = cpool.tile([128, 128], BF16, name="psw")
            nc.gpsimd.dma_start(out=psw, in_=pswap[:, :])
            nc.gpsimd.dma_start(out=cos_t, in_=cosb[:, :])
            nc.gpsimd.dma_start(out=sin_t, in_=sinb[:, :])
            nc.gpsimd.dma_start(out=wv_sb, in_=wv.rearrange("(c p) j -> p c j", p=128))
            nc.gpsimd.dma_start(
                out=wo_sb, in_=wo.rearrange("(c p) (n j) -> p c n j", p=128, n=2))

            # K^T / Q^T pair tiles: [128 dims (head 2p | head 2p+1), S]
            kt_tiles = [kqpool.tile([128, S], BF16, name=f"ktp{p}", tag=f"ktp{p}")
                        for p in range(NP)]
            qt_tiles = [kqpool.tile([128, S], BF16, name=f"qtp{p}", tag=f"qtp{p}")
                        for p in range(NP)]
            # V tiles with ones column: [128 keys, 8 heads, 64+1]
            v_tiles = [vpool.tile([128, 8, HD + 1], BF16, name=f"vt{t}", tag=f"vt{t}")
                       for t in range(NKT)]
            for t in range(NKT):
                nc.gpsimd.memset(v_tiles[t][:, :, HD], 1.0)

            # ctx^T accumulators, one per query block:
            # [128 dims-of-chunk, 4 chunks, 512 tokens]
            ctxT_qb = [kqpool.tile([128, 4, THT], BF16, name=f"ctxT{q}",
                                   tag=f"ctxT{q}") for q in range(NTH)]

            with (
                tc.tile_pool(name="pst", bufs=2, space="PSUM") as pst,
                tc.tile_pool(name="pssc", bufs=2, space="PSUM") as pssc,
                tc.tile_pool(name="psav", bufs=2, space="PSUM") as psav,
            ):
                def oproj(t):
                    # output projection for token tile t (128 tokens)
                    cT = ctxT_qb[t // 4]
                    tl = t % 4
                    for nn in range(2):
                        acc = pst.tile([128, DH], F32, name="oacc", tag="tp")
                        for c in range(4):
                            nc.tensor.matmul(
                                acc, cT[:, c, tl * 128:(tl + 1) * 128],
                                wo_sb[:, c, nn, :], start=(c == 0), stop=(c == 3))
                        osb = spool.tile([128, DH], BF16, name="osb", tag="osb", bufs=2)
                        nc.vector.tensor_copy(osb, acc)
                        nc.sync.dma_start(
                            out=outp[t * 128:(t + 1) * 128, nn * DH:(nn + 1) * DH],
                            in_=osb)

                def do_xT(th):
                    # x^T for pass th via xbar DMA transpose:
                    # out[p, c, t] = x[th*512 + t, 128c + p]
                    xtb = xtpool.tile([128, 8, THT], BF16, name=f"xtb{th}", tag="xt")
                    nc.sync.dma_start_transpose(
                        out=xtb, in_=x[th * THT:(th + 1) * THT, :])
                    return xtb

                xtb = do_xT(0)
                nc.sync.dma_start(out=wk_sb,
                                  in_=wk.rearrange("(c p) j -> p c j", p=128))
                nc.sync.dma_start(out=wq_sb,
                                  in_=wq.rearrange("(c p) j -> p c j", p=128))
                pending_ctxT = []
                for th in range(NTH):
                    t0 = th * THT
                    qb = th
                    q0 = qb * THT
                    nchunk = 2 * qb + 2

                    def do_rope(p):
                        # Q^T/K^T projection + RoPE for head pair p
                        for wsb, dst in ((wk_sb, kt_tiles), (wq_sb, qt_tiles)):
                            acc = pst.tile([128, THT], F32, name="acc", tag="tp")
                            for dc in range(8):
                                nc.tensor.matmul(
                                    acc, wsb[:, dc, p * 128:(p + 1) * 128],
                                    xtb[:, dc, :], start=(dc == 0), stop=(dc == 7))
                            dslice = dst[p][:, t0:t0 + THT]
                            nc.vector.tensor_mul(dslice, acc, cos_t[:, t0:t0 + THT])
                            raw = spool.tile([128, THT], BF16, name="raw", tag="raw",
                                             bufs=2)
                            nc.vector.tensor_mul(raw, acc, sin_t[:, t0:t0 + THT])
                            # r = a*cos - swap32(a*sin); the 32-row block swap
                            # runs on PE via a permutation matmul into PSUM
                            swp = psav.tile([128, THT], F32, name="swp", tag="av")
                            nc.tensor.matmul(swp, psw, raw, start=True, stop=True)
                            nc.vector.tensor_sub(dslice, dslice, swp)

                    def do_vproj():
                        for tl in range(4):
                            acc = pst.tile([128, DH], F32, name="vacc", tag="tp")
                            for dc in range(8):
                                nc.tensor.matmul(
                                    acc, xtb[:, dc, tl * 128:(tl + 1) * 128],
                                    wv_sb[:, dc, :], start=(dc == 0), stop=(dc == 7))
                            vt = v_tiles[th * 4 + tl]
                            nc.vector.tensor_copy(
                                vt[:, :, 0:HD],
                                acc.rearrange("a (h d) -> a h d", h=8))

                    def do_scores(h, j):
                        p, half = h // 2, h % 2
                        r0, r1 = 64 * half, 64 * half + 64
                        qoff = 256 if j == nchunk - 1 else 0
                        sc = pssc.tile([128, 2, THT], F32, name="sc", tag="sc")
                        for s_ in range(2):
                            kt = 2 * j + s_
                            nc.tensor.matmul(
                                sc[:, s_, qoff:THT],
                                kt_tiles[p][r0:r1, kt * 128:(kt + 1) * 128],
                                qt_tiles[p][r0:r1, q0 + qoff:q0 + THT],
                                start=True, stop=True)
                        eab = epool.tile([128, 2, THT], BF16, name="eab",
                                         tag="eab", bufs=20)
                        nc.scalar.activation(
                            eab[:, :, qoff:THT], sc[:, :, qoff:THT],
                            mybir.ActivationFunctionType.Exp, scale=SCALE)
                        if j == nchunk - 2:
                            # diag chunk even: keys rel 0..255 of block
                            nc.gpsimd.affine_select(
                                out=eab[:, 0, 0:128], in_=eab[:, 0, 0:128],
                                compare_op=mybir.AluOpType.is_ge, fill=0.0,
                                base=0, pattern=[[1, 128]],
                                channel_multiplier=-1)
                            nc.gpsimd.affine_select(
                                out=eab[:, 1, 0:256], in_=eab[:, 1, 0:256],
                                compare_op=mybir.AluOpType.is_ge, fill=0.0,
                                base=-128, pattern=[[1, 256]],
                                channel_multiplier=-1)
                        elif j == nchunk - 1:
                            # diag chunk odd: keys rel 256..511, cols 256..511
                            nc.gpsimd.affine_select(
                                out=eab[:, 0, 256:384], in_=eab[:, 0, 256:384],
                                compare_op=mybir.AluOpType.is_ge, fill=0.0,
                                base=0, pattern=[[1, 128]],
                                channel_multiplier=-1)
                            nc.gpsimd.affine_select(
                                out=eab[:, 1, 256:512], in_=eab[:, 1, 256:512],
                                compare_op=mybir.AluOpType.is_ge, fill=0.0,
                                base=-128, pattern=[[1, 256]],
                                channel_multiplier=-1)
                        return eab

                    # normalized ctx staging: [128 queries, 512 dims] per qt
                    csts = [spool.tile([128, DH], BF16, name=f"cst{qt}",
                                       tag=f"cst{qt}", bufs=2) for qt in range(4)]

                    def do_av(h, qt, eabs_h):
                        # AV (flipped): out [128 queries, 65]
                        qt_g = 4 * qb + qt
                        av = psav.tile([128, 128], F32, name="av", tag="av")
                        for kt in range(qt_g + 1):
                            nc.tensor.matmul(
                                av[:, 0:HD + 1],
                                eabs_h[kt // 2][:, kt % 2,
                                                qt * 128:(qt + 1) * 128],
                                v_tiles[kt][:, h, :],
                                start=(kt == 0), stop=(kt == qt_g))
                        rec = spool.tile([128, 1], F32, name="rec", tag="rec",
                                         bufs=2)
                        nc.vector.reciprocal(rec, av[:, HD:HD + 1])
                        nc.vector.tensor_scalar(
                            out=csts[qt][:, HD * h:HD * (h + 1)],
                            in0=av[:, 0:HD], scalar1=rec, scalar2=None,
                            op0=mybir.AluOpType.mult)

                    # software-pipelined pass: rope pairs, V-proj, next-pass
                    # x^T and oproj are spread through the head stream so PE
                    # always has ready work while ACT drains exps
                    do_rope(0)
                    for fn in pending_ctxT:
                        fn()
                    pending_ctxT = []
                    eabs_prev = None
                    for h in range(8):
                        eabs = [do_scores(h, j) for j in range(nchunk)]
                        if h == 0:
                            do_rope(1)
                            do_vproj()
                            if th + 1 < NTH:
                                next_xtb = do_xT(th + 1)
                        elif h == 1:
                            do_rope(2)
                        elif h == 2:
                            do_rope(3)
                        if eabs_prev is not None:
                            for qt in range(4):
                                do_av(h - 1, qt, eabs_prev)
                        # defer output projections to the ACT-bound late passes
                        if th == 2 and h % 2 == 1:
                            oproj(h // 2)
                        elif th == 3:
                            oproj(4 + h)
                        eabs_prev = eabs
                    if th < 3:
                        for qt in range(4):
                            do_av(7, qt, eabs_prev)
                        # ctx^T via xbar DMA: [128 q, 512 d] -> [128, 4, 128]
                        def mk(qb_, csts_):
                            def emit():
                                for qt in range(4):
                                    nc.sync.dma_start_transpose(
                                        out=ctxT_qb[qb_][:, :,
                                                         qt * 128:(qt + 1) * 128],
                                        in_=csts_[qt])
                            return emit
                        pending_ctxT.append(mk(qb, csts))
                        xtb = next_xtb
                    else:
                        # last pass: chain per qt so the tail pipelines
                        for qt in range(4):
                            do_av(7, qt, eabs_prev)
                            nc.sync.dma_start_transpose(
                                out=ctxT_qb[3][:, :, qt * 128:(qt + 1) * 128],
                                in_=csts[qt])
                            oproj(12 + qt)

    nc.compile()
    return nc


def _host_tables(token_positions):
    pos = np.asarray(token_positions, dtype=np.float64)
    inv_freq = np.exp(np.arange(0, HD, 2, dtype=np.float64) * (-math.log(THETA) / HD))
    ang = pos[:, None] * inv_freq[None, :]  # [S, 32]
    cos = np.cos(ang).astype(np.float32).T  # [32, S]
    sin = np.sin(ang).astype(np.float32).T
    C = np.empty((128, S), np.float32)
    Sx = np.empty((128, S), np.float32)
    for half in range(2):
        r = 64 * half
        C[r:r + 32] = cos
        C[r + 32:r + 64] = cos
        Sx[r:r + 32] = -sin
        Sx[r + 32:r + 64] = sin
    return C, Sx


def kernel(in_features, token_positions, wq, wk, wv, wo):
    global _cached
    if _cached is None:
        _cached = _build()
    nc = _cached
    import ml_dtypes
    bf16 = ml_dtypes.bfloat16

    x = np.asarray(in_features, dtype=np.float32)
    perm = np.concatenate(
        [64 * h + np.concatenate([np.arange(0, 64, 2), np.arange(1, 64, 2)])
         for h in range(H)])
    wqp = np.ascontiguousarray(wq[:, perm]).astype(bf16)
    wkp = np.ascontiguousarray(wk[:, perm]).astype(bf16)
    wv = np.asarray(wv, dtype=np.float32).astype(bf16)
    wo = np.asarray(wo, dtype=np.float32).astype(bf16)
    C, Sx = _host_tables(token_positions)
    # 32-row block-swap permutation: out = P @ raw, P[i, swap(i)] = 1,
    # matmul computes lhsT.T @ rhs with lhsT = P^T, so store P^T = P (symmetric)
    P = np.zeros((128, 128), np.float32)
    for i in range(128):
        P[i ^ 32, i] = 1.0
    P = P.astype(bf16)

    in_maps = []
    for c in range(8):
        b, g = c // 2, c % 2
        sl = slice(g * DH, (g + 1) * DH)
        in_maps.append({
            "x": np.ascontiguousarray(x[b]).astype(bf16),
            "wq": np.ascontiguousarray(wqp[:, sl]),
            "wk": np.ascontiguousarray(wkp[:, sl]),
            "wv": np.ascontiguousarray(wv[:, sl]),
            "wo": np.ascontiguousarray(wo[sl, :]),
            "cosb": C,
            "sinb": Sx,
            "pswap": P,
        })
    results = _run(nc, in_maps)
    out = np.empty((B, S, D), np.float32)
    for b in range(B):
        out[b] = (results[2 * b]["outp"].astype(np.float32)
                  + results[2 * b + 1]["outp"].astype(np.float32))
    return out


_jit_cache = None


def _run(nc, in_maps):
    """Run the SPMD program on 8 cores, caching the jitted executable."""
    global _jit_cache
    try:
        import jax
        from jax.sharding import Mesh, PartitionSpec
        from jax.experimental.shard_map import shard_map
        from concourse import bass2jax
        import concourse.mybir as mybir_

        if _jit_cache is None:
            bass2jax.install_neuronx_cc_hook()
            pid_name = nc.partition_id_tensor.name if nc.partition_id_tensor else None
            in_names, out_names, out_avals, zero_outs = [], [], [], []
            for alloc in nc.m.functions[0].allocations:
                if not isinstance(alloc, mybir_.MemoryLocationSet):
                    continue
                nm = alloc.memorylocations[0].name
                if alloc.kind == "ExternalInput":
                    if nm != pid_name:
                        in_names.append(nm)
                elif alloc.kind == "ExternalOutput":
                    out_names.append(nm)
                    shape = tuple(alloc.tensor_shape)
                    dtype = mybir_.dt.np(alloc.dtype)
                    out_avals.append(jax.core.ShapedArray(shape, dtype))
                    zero_outs.append(np.zeros(shape, dtype))
            n_params = len(in_names)
            all_names = in_names + out_names
            if pid_name is not None:
                all_names = all_names + [pid_name]

            def _body(*args):
                operands = list(args)
                if pid_name is not None:
                    operands.append(bass2jax.partition_id_tensor())
                outs = bass2jax._bass_exec_p.bind(
                    *operands, out_avals=tuple(out_avals), in_names=tuple(all_names),
                    out_names=tuple(out_names), lowering_input_output_aliases=(),
                    sim_require_finite=True, sim_require_nnan=True, nc=nc)
                return tuple(outs)

            devices = jax.devices()[:8]
            mesh = Mesh(np.asarray(devices), ("core",))
            nio = n_params + len(out_names)
            sharded = jax.jit(
                shard_map(_body, mesh=mesh, in_specs=(PartitionSpec("core"),) * nio,
                          out_specs=(PartitionSpec("core"),) * len(out_names),
                          check_rep=False),
                keep_unused=True)
            _jit_cache = (sharded, in_names, out_names, zero_outs)

        sharded, in_names, out_names, zero_outs = _jit_cache
        concat_in = [np.concatenate([np.asarray(m[nm]) for m in in_maps], axis=0)
                     for nm in in_names]
        concat_zero = [np.concatenate([z] * 8, axis=0) for z in zero_outs]
        outs = sharded(*concat_in, *concat_zero)
        results = []
        for c in range(8):
            d = {}
            for i, nm in enumerate(out_names):
                arr = np.asarray(outs[i])
                n0 = arr.shape[0] // 8
                d[nm] = arr[c * n0:(c + 1) * n0]
            results.append(d)
        return results
    except Exception:
        res = run_bass_kernel_spmd(nc, in_maps, core_ids=list(range(8)))
        return res.results


# revision 3
# speedup vs baseline: 1.6921x; 1.0542x over previous
"""Causal MHA with RoPE on 8 Trainium2 NeuronCores — v2 (all-bf16, flipped AV).

Sharding: core c -> batch b=c//2, head-group g=c%2 (8 heads of 16).
Each core: Q/K/V projections for its 512 head-dims over the full sequence,
causal attention for its 8 heads, partial output projection (its 512 rows
of wo). Host sums the two partial (bf16) outputs per batch. No collectives.

v2 changes vs baseline:
- All matmul operands bf16 (x, w, q, k, v, exp-scores, ctx): 1.0 cycles/row
  with no fp32r small-tile (ap<256) 4x penalty; transposes 1.0 vs 2.0.
- AV matmul flipped to out[queries, dims]: cost is charged per output
  free-element, so [128q, 65] accumulation over key tiles costs 65/pass
  instead of 128+/pass -> AV drops ~139K -> ~71K PE cycles.
- Denominators land on PSUM partitions (ones-column of V), so softmax
  normalization is reciprocal [128,1] + per-partition tensor_scalar mult.
- Causal masking post-exp via gpsimd affine_select on eab (Pool engine),
  keeping DVE free; exp runs on unmasked scores (finite, bounded).
- Output projection consumes PE-transposed ctx; out DMA'd as bf16 partials.
"""

import math

import numpy as np

import concourse.bass as bass
import concourse.mybir as mybir
import concourse.tile as tile
from concourse import bacc
from concourse.bass_utils import run_bass_kernel_spmd
from concourse.masks import make_identity

F32 = mybir.dt.float32
BF16 = mybir.dt.bfloat16

B, S, D, H = 4, 2048, 1024, 16
HD = D // H          # 64
THETA = 10000.0
DH = D // 2          # 512 per-core head dims (8 heads)
NP = 4               # head pairs per core
NTH = 4              # token passes (512 each)
THT = S // NTH       # 512
NKT = S // 128       # 16 key tiles of 128
SCALE = 1.0 / math.sqrt(HD)

_cached = None


def _build():
    nc = bacc.Bacc(None, target_bir_lowering=False)

    x = nc.dram_tensor("x", [S, D], BF16, kind="ExternalInput")
    wq = nc.dram_tensor("wq", [D, DH], BF16, kind="ExternalInput")
    wk = nc.dram_tensor("wk", [D, DH], BF16, kind="ExternalInput")
    wv = nc.dram_tensor("wv", [D, DH], BF16, kind="ExternalInput")
    wo = nc.dram_tensor("wo", [DH, D], BF16, kind="ExternalInput")
    cosb = nc.dram_tensor("cosb", [128, S], BF16, kind="ExternalInput")
    sinb = nc.dram_tensor("sinb", [128, S], BF16, kind="ExternalInput")
    pswap = nc.dram_tensor("pswap", [128, 128], BF16, kind="ExternalInput")
    outp = nc.dram_tensor("outp", [S, D], BF16, kind="ExternalOutput")

    with tile.TileContext(nc) as tc:
        with (
            tc.tile_pool(name="const", bufs=1) as cpool,
            tc.tile_pool(name="kq", bufs=1) as kqpool,
            tc.tile_pool(name="vaug", bufs=1) as vpool,
            tc.tile_pool(name="xt", bufs=2) as xtpool,
            tc.tile_pool(name="stream", bufs=2) as spool,
            tc.tile_pool(name="eab", bufs=1) as epool,
            tc.tile_pool(name="w0", bufs=1) as wpool,
        ):
            # resident weights; wk/wq on the HWDGE path (needed first),
            # wv/wo + tables on SWDGE in parallel
            wq_sb = wpool.tile([128, 8, DH], BF16, name="wq_sb")
            wk_sb = wpool.tile([128, 8, DH], BF16, name="wk_sb")
            wv_sb = wpool.tile([128, 8, DH], BF16, name="wv_sb")
            wo_sb = wpool.tile([128, 4, 2, DH], BF16, name="wo_sb")
            cos_t = cpool.tile([128, S], BF16, name="cos_t")
            sin_t = cpool.tile([128, S], BF16, name="sin_t")
            psw = cpool.tile([128, 128], BF16, name="psw")
            nc.gpsimd.dma_start(out=psw, in_=pswap[:, :])
            nc.gpsimd.dma_start(out=wv_sb, in_=wv.rearrange("(c p) j -> p c j", p=128))
            nc.gpsimd.dma_start(
                out=wo_sb, in_=wo.rearrange("(c p) (n j) -> p c n j", p=128, n=2))

            # K^T / Q^T pair tiles: [128 dims (head 2p | head 2p+1), S]
            kt_tiles = [kqpool.tile([128, S], BF16, name=f"ktp{p}", tag=f"ktp{p}")
                        for p in range(NP)]
            qt_tiles = [kqpool.tile([128, S], BF16, name=f"qtp{p}", tag=f"qtp{p}")
                        for p in range(NP)]
            # V tiles with ones column: [128 keys, 8 heads, 64+1]
            v_tiles = [vpool.tile([128, 8, HD + 1], BF16, name=f"vt{t}", tag=f"vt{t}")
                       for t in range(NKT)]
            for t in range(NKT):
                nc.gpsimd.memset(v_tiles[t][:, :, HD], 1.0)

            # ctx^T accumulators, one per query block:
            # [128 dims-of-chunk, 4 chunks, 512 tokens]
            ctxT_qb = [kqpool.tile([128, 4, THT], BF16, name=f"ctxT{q}",
                                   tag=f"ctxT{q}") for q in range(NTH)]

            with (
                tc.tile_pool(name="pst", bufs=2, space="PSUM") as pst,
                tc.tile_pool(name="pssc", bufs=2, space="PSUM") as pssc,
                tc.tile_pool(name="psav", bufs=2, space="PSUM") as psav,
            ):
                def oproj(t):
                    # output projection for token tile t (128 tokens)
                    cT = ctxT_qb[t // 4]
                    tl = t % 4
                    for nn in range(2):
                        acc = pst.tile([128, DH], F32, name="oacc", tag="tp")
                        for c in range(4):
                            nc.tensor.matmul(
                                acc, cT[:, c, tl * 128:(tl + 1) * 128],
                                wo_sb[:, c, nn, :], start=(c == 0), stop=(c == 3))
                        osb = spool.tile([128, DH], BF16, name="osb", tag="osb", bufs=2)
                        nc.vector.tensor_copy(osb, acc)
                        nc.sync.dma_start(
                            out=outp[t * 128:(t + 1) * 128, nn * DH:(nn + 1) * DH],
                            in_=osb)

                def do_xT(th):
                    # x^T for pass th via xbar DMA transpose:
                    # out[p, c, t] = x[th*512 + t, 128c + p]
                    xtb = xtpool.tile([128, 8, THT], BF16, name=f"xtb{th}", tag="xt")
                    nc.sync.dma_start_transpose(
                        out=xtb, in_=x[th * THT:(th + 1) * THT, :])
                    return xtb

                xtb = do_xT(0)
                nc.sync.dma_start(out=wk_sb,
                                  in_=wk.rearrange("(c p) j -> p c j", p=128))
                nc.sync.dma_start(out=cos_t[:, 0:THT], in_=cosb[:, 0:THT])
                nc.sync.dma_start(out=sin_t[:, 0:THT], in_=sinb[:, 0:THT])
                nc.sync.dma_start(out=wq_sb,
                                  in_=wq.rearrange("(c p) j -> p c j", p=128))
                for c_ in range(1, NTH):
                    nc.sync.dma_start(out=cos_t[:, c_ * THT:(c_ + 1) * THT],
                                      in_=cosb[:, c_ * THT:(c_ + 1) * THT])
                    nc.sync.dma_start(out=sin_t[:, c_ * THT:(c_ + 1) * THT],
                                      in_=sinb[:, c_ * THT:(c_ + 1) * THT])
                pending_ctxT = []
                for th in range(NTH):
                    t0 = th * THT
                    qb = th
                    q0 = qb * THT
                    nchunk = 2 * qb + 2

                    def do_rope(p, xtb_=None, t0_=None):
                        # Q^T/K^T projection + RoPE for head pair p
                        xtb_ = xtb if xtb_ is None else xtb_
                        t0_ = t0 if t0_ is None else t0_
                        for wsb, dst in ((wk_sb, kt_tiles), (wq_sb, qt_tiles)):
                            acc = pst.tile([128, THT], F32, name="acc", tag="tp")
                            for dc in range(8):
                                nc.tensor.matmul(
                                    acc, wsb[:, dc, p * 128:(p + 1) * 128],
                                    xtb_[:, dc, :], start=(dc == 0), stop=(dc == 7))
                            dslice = dst[p][:, t0_:t0_ + THT]
                            nc.vector.tensor_mul(dslice, acc, cos_t[:, t0_:t0_ + THT])
                            raw = spool.tile([128, THT], BF16, name="raw", tag="raw",
                                             bufs=2)
                            nc.vector.tensor_mul(raw, acc, sin_t[:, t0_:t0_ + THT])
                            # r = a*cos - swap32(a*sin); the 32-row block swap
                            # runs on PE via a permutation matmul into PSUM
                            swp = psav.tile([128, THT], F32, name="swp", tag="av")
                            nc.tensor.matmul(swp, psw, raw, start=True, stop=True)
                            nc.vector.tensor_sub(dslice, dslice, swp)

                    def do_vproj():
                        for tl in range(4):
                            acc = pst.tile([128, DH], F32, name="vacc", tag="tp")
                            for dc in range(8):
                                nc.tensor.matmul(
                                    acc, xtb[:, dc, tl * 128:(tl + 1) * 128],
                                    wv_sb[:, dc, :], start=(dc == 0), stop=(dc == 7))
                            vt = v_tiles[th * 4 + tl]
                            nc.vector.tensor_copy(
                                vt[:, :, 0:HD],
                                acc.rearrange("a (h d) -> a h d", h=8))

                    def do_scores(h, j):
                        p, half = h // 2, h % 2
                        r0, r1 = 64 * half, 64 * half + 64
                        qoff = 256 if j == nchunk - 1 else 0
                        sc = pssc.tile([128, 2, THT], F32, name="sc", tag="sc")
                        for s_ in range(2):
                            kt = 2 * j + s_
                            nc.tensor.matmul(
                                sc[:, s_, qoff:THT],
                                kt_tiles[p][r0:r1, kt * 128:(kt + 1) * 128],
                                qt_tiles[p][r0:r1, q0 + qoff:q0 + THT],
                                start=True, stop=True)
                        eab = epool.tile([128, 2, THT], BF16, name="eab",
                                         tag="eab", bufs=20)
                        nc.scalar.activation(
                            eab[:, :, qoff:THT], sc[:, :, qoff:THT],
                            mybir.ActivationFunctionType.Exp, scale=SCALE)
                        if j == nchunk - 2:
                            # diag chunk even: keys rel 0..255 of block
                            nc.gpsimd.affine_select(
                                out=eab[:, 0, 0:128], in_=eab[:, 0, 0:128],
                                compare_op=mybir.AluOpType.is_ge, fill=0.0,
                                base=0, pattern=[[1, 128]],
                                channel_multiplier=-1)
                            nc.gpsimd.affine_select(
                                out=eab[:, 1, 0:256], in_=eab[:, 1, 0:256],
                                compare_op=mybir.AluOpType.is_ge, fill=0.0,
                                base=-128, pattern=[[1, 256]],
                                channel_multiplier=-1)
                        elif j == nchunk - 1:
                            # diag chunk odd: keys rel 256..511, cols 256..511
                            nc.gpsimd.affine_select(
                                out=eab[:, 0, 256:384], in_=eab[:, 0, 256:384],
                                compare_op=mybir.AluOpType.is_ge, fill=0.0,
                                base=0, pattern=[[1, 128]],
                                channel_multiplier=-1)
                            nc.gpsimd.affine_select(
                                out=eab[:, 1, 256:512], in_=eab[:, 1, 256:512],
                                compare_op=mybir.AluOpType.is_ge, fill=0.0,
                                base=-128, pattern=[[1, 256]],
                                channel_multiplier=-1)
                        return eab

                    # normalized ctx staging: [128 queries, 512 dims] per qt
                    csts = [spool.tile([128, DH], BF16, name=f"cst{qt}",
                                       tag=f"cst{qt}", bufs=2) for qt in range(4)]

                    def do_av(h, qt, eabs_h):
                        # AV (flipped): out [128 queries, 65]
                        qt_g = 4 * qb + qt
                        av = psav.tile([128, 128], F32, name="av", tag="av")
                        for kt in range(qt_g + 1):
                            nc.tensor.matmul(
                                av[:, 0:HD + 1],
                                eabs_h[kt // 2][:, kt % 2,
                                                qt * 128:(qt + 1) * 128],
                                v_tiles[kt][:, h, :],
                                start=(kt == 0), stop=(kt == qt_g))
                        rec = spool.tile([128, 1], F32, name="rec", tag="rec",
                                         bufs=2)
                        nc.vector.reciprocal(rec, av[:, HD:HD + 1])
                        nc.vector.tensor_scalar(
                            out=csts[qt][:, HD * h:HD * (h + 1)],
                            in0=av[:, 0:HD], scalar1=rec, scalar2=None,
                            op0=mybir.AluOpType.mult)

                    # software-pipelined pass: rope pairs, V-proj, next-pass
                    # x^T and oproj are spread through the head stream so PE
                    # always has ready work while ACT drains exps
                    if th == 0:
                        do_rope(0)
                    for fn in pending_ctxT:
                        fn()
                    pending_ctxT = []
                    eabs_prev = None
                    for h in range(8):
                        eabs = [do_scores(h, j) for j in range(nchunk)]
                        if h == 0:
                            if th == 0:
                                do_rope(1)
                            do_vproj()
                            if th + 1 < NTH:
                                next_xtb = do_xT(th + 1)
                        elif h == 1 and th == 0:
                            do_rope(2)
                        elif h == 2 and th == 0:
                            do_rope(3)
                        elif 3 <= h <= 6 and th + 1 < NTH:
                            do_rope(h - 3, xtb_=next_xtb, t0_=(th + 1) * THT)
                        if eabs_prev is not None:
                            for qt in range(4):
                                do_av(h - 1, qt, eabs_prev)
                        # defer output projections to the ACT-bound last pass
                        if th == 3:
                            oproj(h)
                            if h % 2 == 1:
                                oproj(8 + h // 2)
                        eabs_prev = eabs
                    if th < 3:
                        for qt in range(4):
                            do_av(7, qt, eabs_prev)
                        # ctx^T via xbar DMA: [128 q, 512 d] -> [128, 4, 128]
                        def mk(qb_, csts_):
                            def emit():
                                for qt in range(4):
                                    nc.sync.dma_start_transpose(
                                        out=ctxT_qb[qb_][:, :,
                                                         qt * 128:(qt + 1) * 128],
                                        in_=csts_[qt])
                            return emit
                        pending_ctxT.append(mk(qb, csts))
                        xtb = next_xtb
                    else:
                        # last pass: av chain + its ctx transpose per qt, then
                        # the remaining output projections
                        for qt in range(4):
                            do_av(7, qt, eabs_prev)
                            nc.sync.dma_start_transpose(
                                out=ctxT_qb[3][:, :, qt * 128:(qt + 1) * 128],
                                in_=csts[qt])
                        for qt in range(4):
                            oproj(12 + qt)

    nc.compile()
    return nc


def _host_tables(token_positions):
    pos = np.asarray(token_positions, dtype=np.float64)
    inv_freq = np.exp(np.arange(0, HD, 2, dtype=np.float64) * (-math.log(THETA) / HD))
    ang = pos[:, None] * inv_freq[None, :]  # [S, 32]
    cos = np.cos(ang).astype(np.float32).T  # [32, S]
    sin = np.sin(ang).astype(np.float32).T
    C = np.empty((128, S), np.float32)
    Sx = np.empty((128, S), np.float32)
    for half in range(2):
        r = 64 * half
        C[r:r + 32] = cos
        C[r + 32:r + 64] = cos
        Sx[r:r + 32] = -sin
        Sx[r + 32:r + 64] = sin
    return C, Sx


def kernel(in_features, token_positions, wq, wk, wv, wo):
    global _cached
    if _cached is None:
        _cached = _build()
    nc = _cached
    import ml_dtypes
    bf16 = ml_dtypes.bfloat16

    x = np.asarray(in_features, dtype=np.float32)
    perm = np.concatenate(
        [64 * h + np.concatenate([np.arange(0, 64, 2), np.arange(1, 64, 2)])
         for h in range(H)])
    wqp = np.ascontiguousarray(wq[:, perm]).astype(bf16)
    wkp = np.ascontiguousarray(wk[:, perm]).astype(bf16)
    wv = np.asarray(wv, dtype=np.float32).astype(bf16)
    wo = np.asarray(wo, dtype=np.float32).astype(bf16)
    C, Sx = _host_tables(token_positions)
    C = C.astype(bf16)
    Sx = Sx.astype(bf16)
    # 32-row block-swap permutation: out = P @ raw, P[i, swap(i)] = 1,
    # matmul computes lhsT.T @ rhs with lhsT = P^T, so store P^T = P (symmetric)
    P = np.zeros((128, 128), np.float32)
    for i in range(128):
        P[i ^ 32, i] = 1.0
    P = P.astype(bf16)

    in_maps = []
    for c in range(8):
        b, g = c // 2, c % 2
        sl = slice(g * DH, (g + 1) * DH)
        in_maps.append({
            "x": np.ascontiguousarray(x[b]).astype(bf16),
            "wq": np.ascontiguousarray(wqp[:, sl]),
            "wk": np.ascontiguousarray(wkp[:, sl]),
            "wv": np.ascontiguousarray(wv[:, sl]),
            "wo": np.ascontiguousarray(wo[sl, :]),
            "cosb": C,
            "sinb": Sx,
            "pswap": P,
        })
    results = _run(nc, in_maps)
    out = np.empty((B, S, D), np.float32)
    for b in range(B):
        out[b] = (results[2 * b]["outp"].astype(np.float32)
                  + results[2 * b + 1]["outp"].astype(np.float32))
    return out


_jit_cache = None


def _run(nc, in_maps):
    """Run the SPMD program on 8 cores, caching the jitted executable."""
    global _jit_cache
    try:
        import jax
        from jax.sharding import Mesh, PartitionSpec
        from jax.experimental.shard_map import shard_map
        from concourse import bass2jax
        import concourse.mybir as mybir_

        if _jit_cache is None:
            bass2jax.install_neuronx_cc_hook()
            pid_name = nc.partition_id_tensor.name if nc.partition_id_tensor else None
            in_names, out_names, out_avals, zero_outs = [], [], [], []
            for alloc in nc.m.functions[0].allocations:
                if not isinstance(alloc, mybir_.MemoryLocationSet):
                    continue
                nm = alloc.memorylocations[0].name
                if alloc.kind == "ExternalInput":
                    if nm != pid_name:
                        in_names.append(nm)
                elif alloc.kind == "ExternalOutput":
                    out_names.append(nm)
                    shape = tuple(alloc.tensor_shape)
                    dtype = mybir_.dt.np(alloc.dtype)
                    out_avals.append(jax.core.ShapedArray(shape, dtype))
                    zero_outs.append(np.zeros(shape, dtype))
            n_params = len(in_names)
            all_names = in_names + out_names
            if pid_name is not None:
                all_names = all_names + [pid_name]

            def _body(*args):
                operands = list(args)
                if pid_name is not None:
                    operands.append(bass2jax.partition_id_tensor())
                outs = bass2jax._bass_exec_p.bind(
                    *operands, out_avals=tuple(out_avals), in_names=tuple(all_names),
                    out_names=tuple(out_names), lowering_input_output_aliases=(),
                    sim_require_finite=True, sim_require_nnan=True, nc=nc)
                return tuple(outs)

            devices = jax.devices()[:8]
            mesh = Mesh(np.asarray(devices), ("core",))
            nio = n_params + len(out_names)
            sharded = jax.jit(
                shard_map(_body, mesh=mesh, in_specs=(PartitionSpec("core"),) * nio,
                          out_specs=(PartitionSpec("core"),) * len(out_names),
                          check_rep=False),
                keep_unused=True)
            _jit_cache = (sharded, in_names, out_names, zero_outs)

        sharded, in_names, out_names, zero_outs = _jit_cache
        concat_in = [np.concatenate([np.asarray(m[nm]) for m in in_maps], axis=0)
                     for nm in in_names]
        concat_zero = [np.concatenate([z] * 8, axis=0) for z in zero_outs]
        outs = sharded(*concat_in, *concat_zero)
        results = []
        for c in range(8):
            d = {}
            for i, nm in enumerate(out_names):
                arr = np.asarray(outs[i])
                n0 = arr.shape[0] // 8
                d[nm] = arr[c * n0:(c + 1) * n0]
            results.append(d)
        return results
    except Exception:
        res = run_bass_kernel_spmd(nc, in_maps, core_ids=list(range(8)))
        return res.results
